# revision 11
# baseline (speedup 1.0000x reference)
"""Trainium2 Bass kernel for nn_AsymmetricLossCustom (8 NeuronCores).

Math (reference):
    s  = sigmoid(x)
    t  = min(1 - s + 0.05, 1)
    loss = y*ln(max(s,eps)) + (1-y)*ln(max(t,eps))        # [B, C]
    active[b,c] = OR_g ( (any_g[b] & ~has_g[b]) & mask_g[c] )
    out = -(loss * where(active, 0.1, 1.0)).sum()

Device scheme (2 ScalarE + 1 GpSimd + 1 GpSimd + 1 VectorE pass / element):
    sp = sigmoid(-x)                     # ACT (sigmoid table set)
    t  = min(sp + 0.05, 1)               # GpSimd tensor_scalar dual-op
    c  = 1 - sp            ( = s )       # GpSimd tensor_scalar dual-op
    w  = y ? c : t                       # DVE copy_predicated (int32 mask view)
    loss = ln(w)                         # ACT (natural_log set), fused
                                         #   accum_out => per-row sum(loss)

Sigmoid and ln live in different ACT table sets, so chunks are processed in
groups: all sigmoids of a group, then all lns — 2 table loads per group
instead of 2 per chunk.

The `active` down-weighting only touches columns appearing in one of the
three index arrays (<=170 of 9605). The host gathers those columns and
builds the 0/1 `active` matrix (pure index/mask logic); the device computes
the loss values for the gathered columns and accumulates
    corr_neg = sum(active * (-loss))
so that  out = -sum(loss) - 0.9 * corr_neg.

Sharding: pure data parallel over the batch. Each core gets 512 rows,
viewed as [128 partitions, 38420 free] (4 rows per partition, contiguous),
plus gathered columns [512, 176]. Host sums the 8 per-core [128, 2]
partials:  result = -sum(out[:,0]) - 0.9 * sum(out[:,1]).
"""

import sys

import numpy as np

if "/opt/trn_rl_repo" not in sys.path:
    sys.path.insert(0, "/opt/trn_rl_repo")

B, C = 4096, 9605
NCORES = 8
ROWS = B // NCORES          # 512 rows per core
P = 128                     # SBUF partitions
FREE = (ROWS // P) * C      # 38420 f32 per partition
NCHUNK = 17
F = FREE // NCHUNK          # 2260 (even: enables DVE 4x mode on bf16 TS ops)
GROUPS = [range(0, 6), range(6, 12), range(12, 17)]
WBUFS = 6                   # covers one phase group
NBT = ROWS // P             # 4 gathered batch-tiles per core
U_PAD = 176                 # padded union-column count (>= 70+70+30)
CLIP = 0.05
ALPHA = 0.1

TRACE = False               # set True (e.g. from test.py) to capture an NTFF profile
LAST_RESULTS = None         # BassKernelResults of the most recent run

_NC = None


def _build_program():
    import concourse.bacc as bacc
    import concourse.mybir as mybir
    from concourse import tile

    from concourse.tile import add_dep_helper

    f32 = mybir.dt.float32
    bf16 = mybir.dt.bfloat16
    i32 = mybir.dt.int32
    Alu = mybir.AluOpType
    Act = mybir.ActivationFunctionType
    AX = mybir.AxisListType

    # Force the ACT engine to execute activations in emission order —
    # otherwise the Tile scheduler interleaves sigmoid and ln chunks and
    # the compiler inserts an ACT_TABLE_LOAD (~1.3us) before nearly every
    # activation instead of one per phase.
    _prev_act = [None]

    def act_order(bi):
        if _prev_act[0] is not None:
            add_dep_helper(bi.ins, _prev_act[0].ins, sync=True,
                           reason="act table-set phase order")
        _prev_act[0] = bi
        return bi

    nc = bacc.Bacc(
        "TRN2",
        target_bir_lowering=False,
        debug=False,
        enable_asserts=False,
        num_devices=NCORES,
    )

    x = nc.dram_tensor("x", [P, FREE], f32, kind="ExternalInput").ap()
    y = nc.dram_tensor("y", [P, FREE], f32, kind="ExternalInput").ap()
    xg = nc.dram_tensor("xg", [ROWS, U_PAD], f32, kind="ExternalInput").ap()
    yg = nc.dram_tensor("yg", [ROWS, U_PAD], f32, kind="ExternalInput").ap()
    avg = nc.dram_tensor("avg", [ROWS, U_PAD], f32, kind="ExternalInput").ap()
    out = nc.dram_tensor("out", [P, 2], f32, kind="ExternalOutput").ap()

    with tile.TileContext(nc) as tc:
        with (
            tc.tile_pool(name="xp", bufs=5) as xp,
            tc.tile_pool(name="yp", bufs=6) as yp,
            tc.tile_pool(name="sp", bufs=2) as sp,
            tc.tile_pool(name="wp", bufs=WBUFS) as wp,
            tc.tile_pool(name="cp", bufs=2) as cp,
            tc.tile_pool(name="lp", bufs=2) as lp,
            tc.tile_pool(name="accp", bufs=1) as accp,
            tc.tile_pool(name="gp", bufs=3) as gp,
            tc.tile_pool(name="finp", bufs=1) as finp,
        ):
            accLW = accp.tile([P, NCHUNK], f32, tag="accLW")
            accC = accp.tile([P, NBT], f32, tag="accC")

            g_y, g_s, g_w, g_av = [], [], [], []

            for gi, grp in enumerate(GROUPS):
                # ---- DMA + sigmoid phase -------------------------------
                yts, sts = {}, {}
                for k in grp:
                    cs = slice(k * F, (k + 1) * F)
                    xt = xp.tile([P, F], f32, tag="x")
                    nc.sync.dma_start(xt[:], x[:, cs])
                    yt = yp.tile([P, F], f32, tag="y")
                    nc.sync.dma_start(yt[:], y[:, cs])
                    st = sp.tile([P, F], f32, tag="s")
                    act_order(nc.scalar.activation(st[:], xt[:], Act.Sigmoid,
                                                   scale=-1.0))
                    yts[k], sts[k] = yt, st
                if gi == 0:
                    for bt in range(NBT):
                        rs = slice(bt * P, (bt + 1) * P)
                        xgt = gp.tile([P, U_PAD], f32, tag="xg")
                        nc.sync.dma_start(xgt[:], xg[rs, :])
                        ygt = gp.tile([P, U_PAD], f32, tag="yg")
                        nc.sync.dma_start(ygt[:], yg[rs, :])
                        avt = gp.tile([P, U_PAD], f32, tag="av")
                        nc.sync.dma_start(avt[:], avg[rs, :])
                        sgt = gp.tile([P, U_PAD], f32, tag="sg")
                        act_order(nc.scalar.activation(sgt[:], xgt[:],
                                                       Act.Sigmoid, scale=-1.0))
                        g_y.append(ygt)
                        g_av.append(avt)
                        g_s.append(sgt)

                # ---- blend phase (GpSimd + DVE) ------------------------
                wts = {}
                for k in grp:
                    st, yt = sts[k], yts[k]
                    wt = wp.tile([P, F], bf16, tag="w")
                    nc.vector.tensor_scalar(wt[:], st[:], CLIP, 1.0,
                                            Alu.add, Alu.min)
                    ct = cp.tile([P, F], bf16, tag="c")
                    nc.vector.tensor_scalar(ct[:], st[:], -1.0, 1.0,
                                            Alu.mult, Alu.add)
                    nc.vector.copy_predicated(
                        wt[:], yt[:].bitcast(i32), ct[:])
                    wts[k] = wt
                if gi == 0:
                    for bt in range(NBT):
                        sgt, ygt = g_s[bt], g_y[bt]
                        wgt = gp.tile([P, U_PAD], f32, tag="wg")
                        nc.vector.tensor_scalar(wgt[:], sgt[:], CLIP, 1.0,
                                                Alu.add, Alu.min)
                        cgt = gp.tile([P, U_PAD], f32, tag="cg")
                        nc.vector.tensor_scalar(cgt[:], sgt[:], -1.0, 1.0,
                                                Alu.mult, Alu.add)
                        nc.vector.copy_predicated(
                            wgt[:], ygt[:].bitcast(i32), cgt[:])
                        g_w.append(wgt)

                # ---- Ln phase ------------------------------------------
                for k in grp:
                    wt = wts[k]
                    lt = lp.tile([P, F], bf16, tag="lt")
                    act_order(nc.scalar.activation(
                        lt[:], wt[:], Act.Ln, accum_out=accLW[:, k : k + 1]))
                if gi == 0:
                    for bt in range(NBT):
                        lgt = gp.tile([P, U_PAD], f32, tag="lg")
                        act_order(nc.scalar.activation(lgt[:], g_w[bt][:],
                                                       Act.Ln))
                        # accC[:,bt] = sum(-active * loss)
                        ja = gp.tile([P, U_PAD], f32, tag="junk")
                        nc.vector.scalar_tensor_tensor(
                            ja[:], g_av[bt][:], -1.0, lgt[:],
                            Alu.mult, Alu.mult,
                            accum_out=accC[:, bt : bt + 1],
                        )

            # ---- final combine -> out [P, 2] ---------------------------
            lossr = finp.tile([P, 1], f32, tag="lossr")
            nc.vector.tensor_reduce(lossr[:], accLW[:], AX.X, Alu.add)
            corrr = finp.tile([P, 1], f32, tag="corrr")
            nc.vector.tensor_reduce(corrr[:], accC[:], AX.X, Alu.add)

            osb = finp.tile([P, 2], f32, tag="osb")
            nc.vector.tensor_copy(out=osb[:, 0:1], in_=lossr[:])
            nc.vector.tensor_copy(out=osb[:, 1:2], in_=corrr[:])
            nc.sync.dma_start(out[:], osb[:])

    nc.compile()
    return nc


def _get_nc():
    global _NC
    if _NC is None:
        _NC = _build_program()
    return _NC


def _ensure_ntff_hook():
    """Register the axon NTFF profile hook if the image's antenv lacks it."""
    import contextlib
    import ctypes
    import types

    try:
        from antenv.axon_hooks import get_axon_ntff_profile_hook  # noqa: F401
        return
    except ImportError:
        pass

    so_path = "/opt/axon/libaxon_pjrt.so"
    try:
        lib = ctypes.CDLL(so_path)
    except OSError:
        return
    if not hasattr(lib, "axon_start_nrt_profile"):
        return
    lib.axon_start_nrt_profile.argtypes = [
        ctypes.POINTER(ctypes.c_int64),
        ctypes.c_size_t,
    ]
    lib.axon_start_nrt_profile.restype = ctypes.c_int64
    lib.axon_stop_nrt_profile.argtypes = [ctypes.c_char_p]
    lib.axon_stop_nrt_profile.restype = ctypes.c_int64

    @contextlib.contextmanager
    def _hook(output_dir, device_ids):
        import jax

        jax.devices()
        if device_ids:
            ids = (ctypes.c_int64 * len(device_ids))(*device_ids)
            rc = lib.axon_start_nrt_profile(ids, len(device_ids))
        else:
            rc = lib.axon_start_nrt_profile(None, 0)
        if rc != 0:
            raise RuntimeError(f"axon_start_nrt_profile rc={rc}")
        try:
            yield
        finally:
            n = lib.axon_stop_nrt_profile(str(output_dir).encode())
            print(f"ntff profile: {n} file(s) written to {output_dir}",
                  file=sys.stderr)

    mod = types.ModuleType("antenv.axon_hooks")
    mod.get_axon_ntff_profile_hook = lambda: _hook
    mod.set_axon_ntff_profile_hook = lambda h: None
    sys.modules["antenv.axon_hooks"] = mod


def kernel(x, y, recycle_ind, donate_ind, compost_ind):
    global LAST_RESULTS
    import concourse.bass_utils as bass_utils

    # Avoid any network artifact upload in the (optional) trace path.
    bass_utils.upload_artifacts = lambda tmpdir: "local://" + tmpdir
    _ensure_ntff_hook()

    x = np.ascontiguousarray(x, dtype=np.float32)
    y = np.ascontiguousarray(y, dtype=np.float32)
    recycle_ind = np.asarray(recycle_ind).astype(np.int64)
    donate_ind = np.asarray(donate_ind).astype(np.int64)
    compost_ind = np.asarray(compost_ind).astype(np.int64)

    # Union of group columns, padded to the fixed program width. Pad
    # columns carry active=0 so they contribute nothing to the correction.
    cols = np.unique(np.concatenate([recycle_ind, donate_ind, compost_ind]))
    u = len(cols)
    assert u <= U_PAD, (u, U_PAD)
    colsp = np.concatenate([cols, np.zeros(U_PAD - u, dtype=cols.dtype)])

    def mask_v(ind):
        v = np.zeros(U_PAD, np.float32)
        v[:u] = np.isin(cols, ind).astype(np.float32)
        return v

    mrv = mask_v(recycle_ind)
    mdv = mask_v(donate_ind)
    mcv = mask_v(compost_ind)

    xg = np.ascontiguousarray(x[:, colsp])
    yg = np.ascontiguousarray(y[:, colsp])

    # active[b, j] from the group masks and per-row has-group flags
    has_r = (yg * mrv).sum(axis=1) > 0
    has_d = (yg * mdv).sum(axis=1) > 0
    has_c = (yg * mcv).sum(axis=1) > 0
    any_g = has_r | has_d | has_c
    a_r = (any_g & ~has_r).astype(np.float32)
    a_d = (any_g & ~has_d).astype(np.float32)
    a_c = (any_g & ~has_c).astype(np.float32)
    av = np.minimum(
        a_r[:, None] * mrv + a_d[:, None] * mdv + a_c[:, None] * mcv, 1.0
    ).astype(np.float32)

    nc = _get_nc()

    in_maps = []
    for i in range(NCORES):
        rs = slice(i * ROWS, (i + 1) * ROWS)
        in_maps.append({
            "x": x[rs].reshape(P, FREE),
            "y": y[rs].reshape(P, FREE),
            "xg": xg[rs],
            "yg": yg[rs],
            "avg": av[rs],
        })

    res = bass_utils.run_bass_kernel_spmd(
        nc, in_maps, core_ids=list(range(NCORES)), trace=TRACE
    )
    LAST_RESULTS = res

    loss_sum = 0.0
    corr_neg = 0.0
    for r in res.results:
        o = r["out"].astype(np.float64)
        loss_sum += o[:, 0].sum()
        corr_neg += o[:, 1].sum()

    total = -loss_sum - (1.0 - ALPHA) * corr_neg
    return np.asarray(total, dtype=np.float32)


# revision 12
# speedup vs baseline: 1.2825x; 1.2825x over previous
"""Trainium2 Bass kernel for nn_AsymmetricLossCustom (8 NeuronCores).

Math (reference):
    s  = sigmoid(x)
    t  = min(1 - s + 0.05, 1)
    loss = y*ln(max(s,eps)) + (1-y)*ln(max(t,eps))        # [B, C]
    active[b,c] = OR_g ( (any_g[b] & ~has_g[b]) & mask_g[c] )
    out = -(loss * where(active, 0.1, 1.0)).sum()

Device scheme (2 ScalarE + 1 GpSimd + 1 GpSimd + 1 VectorE pass / element):
    sp = sigmoid(-x)                     # ACT (sigmoid table set)
    t  = min(sp + 0.05, 1)               # GpSimd tensor_scalar dual-op
    c  = 1 - sp            ( = s )       # GpSimd tensor_scalar dual-op
    w  = y ? c : t                       # DVE copy_predicated (int32 mask view)
    loss = ln(w)                         # ACT (natural_log set), fused
                                         #   accum_out => per-row sum(loss)

Sigmoid and ln live in different ACT table sets, so chunks are processed in
groups: all sigmoids of a group, then all lns — 2 table loads per group
instead of 2 per chunk.

The `active` down-weighting only touches columns appearing in one of the
three index arrays (<=170 of 9605). The host gathers those columns and
builds the 0/1 `active` matrix (pure index/mask logic); the device computes
the loss values for the gathered columns and accumulates
    corr_neg = sum(active * (-loss))
so that  out = -sum(loss) - 0.9 * corr_neg.

Sharding: pure data parallel over the batch. Each core gets 512 rows,
viewed as [128 partitions, 38420 free] (4 rows per partition, contiguous),
plus gathered columns [512, 176]. Host sums the 8 per-core [128, 2]
partials:  result = -sum(out[:,0]) - 0.9 * sum(out[:,1]).
"""

import sys

import numpy as np

if "/opt/trn_rl_repo" not in sys.path:
    sys.path.insert(0, "/opt/trn_rl_repo")

B, C = 4096, 9605
NCORES = 8
ROWS = B // NCORES          # 512 rows per core
P = 128                     # SBUF partitions
FREE = (ROWS // P) * C      # 38420 f32 per partition
NCHUNK = 17
F = FREE // NCHUNK          # 2260 (even: enables DVE 4x mode on bf16 TS ops)
GROUPS = [range(0, 6), range(6, 12), range(12, 17)]
WBUFS = 6                   # covers one phase group
NBT = ROWS // P             # 4 gathered batch-tiles per core
U_PAD = 176                 # padded union-column count (>= 70+70+30)
CLIP = 0.05
ALPHA = 0.1

TRACE = False               # set True (e.g. from test.py) to capture an NTFF profile
LAST_RESULTS = None         # BassKernelResults of the most recent run

_NC = None


def _build_program():
    import concourse.bacc as bacc
    import concourse.mybir as mybir
    from concourse import tile

    from concourse.tile import add_dep_helper

    f32 = mybir.dt.float32
    f16 = mybir.dt.float16
    i32 = mybir.dt.int32
    Alu = mybir.AluOpType
    Act = mybir.ActivationFunctionType
    AX = mybir.AxisListType

    # Force the ACT engine to execute activations in emission order —
    # otherwise the Tile scheduler interleaves sigmoid and ln chunks and
    # the compiler inserts an ACT_TABLE_LOAD (~1.3us) before nearly every
    # activation instead of one per phase.
    _prev_act = [None]

    def act_order(bi):
        if _prev_act[0] is not None:
            add_dep_helper(bi.ins, _prev_act[0].ins, sync=True,
                           reason="act table-set phase order")
        _prev_act[0] = bi
        return bi

    nc = bacc.Bacc(
        "TRN2",
        target_bir_lowering=False,
        debug=False,
        enable_asserts=False,
        num_devices=NCORES,
    )

    x = nc.dram_tensor("x", [P, FREE], f32, kind="ExternalInput").ap()
    y = nc.dram_tensor("y", [P, FREE], f32, kind="ExternalInput").ap()
    xg = nc.dram_tensor("xg", [ROWS, U_PAD], f32, kind="ExternalInput").ap()
    yg = nc.dram_tensor("yg", [ROWS, U_PAD], f32, kind="ExternalInput").ap()
    avg = nc.dram_tensor("avg", [ROWS, U_PAD], f32, kind="ExternalInput").ap()
    out = nc.dram_tensor("out", [P, 2], f32, kind="ExternalOutput").ap()

    with tile.TileContext(nc) as tc:
        with (
            tc.tile_pool(name="xp", bufs=6) as xp,
            tc.tile_pool(name="yp", bufs=6) as yp,
            tc.tile_pool(name="sp", bufs=2) as sp,
            tc.tile_pool(name="wp", bufs=WBUFS) as wp,
            tc.tile_pool(name="cp", bufs=2) as cp,
            tc.tile_pool(name="lp", bufs=2) as lp,
            tc.tile_pool(name="accp", bufs=1) as accp,
            tc.tile_pool(name="gp", bufs=3) as gp,
            tc.tile_pool(name="finp", bufs=1) as finp,
        ):
            accLW = accp.tile([P, NCHUNK], f32, tag="accLW")
            accC = accp.tile([P, NBT], f32, tag="accC")

            g_y, g_s, g_w, g_av = [], [], [], []

            for gi, grp in enumerate(GROUPS):
                # ---- DMA + sigmoid phase -------------------------------
                yts, sts = {}, {}
                for k in grp:
                    cs = slice(k * F, (k + 1) * F)
                    xt = xp.tile([P, F], f32, tag="x")
                    nc.sync.dma_start(xt[:], x[:, cs])
                    yt = yp.tile([P, F], f32, tag="y")
                    nc.sync.dma_start(yt[:], y[:, cs])
                    st = sp.tile([P, F], f16, tag="s")
                    act_order(nc.scalar.activation(st[:], xt[:], Act.Sigmoid,
                                                   scale=-1.0))
                    yts[k], sts[k] = yt, st
                if gi == 0:
                    for bt in range(NBT):
                        rs = slice(bt * P, (bt + 1) * P)
                        xgt = gp.tile([P, U_PAD], f32, tag="xg")
                        nc.sync.dma_start(xgt[:], xg[rs, :])
                        ygt = gp.tile([P, U_PAD], f32, tag="yg")
                        nc.sync.dma_start(ygt[:], yg[rs, :])
                        avt = gp.tile([P, U_PAD], f32, tag="av")
                        nc.sync.dma_start(avt[:], avg[rs, :])
                        sgt = gp.tile([P, U_PAD], f32, tag="sg")
                        act_order(nc.scalar.activation(sgt[:], xgt[:],
                                                       Act.Sigmoid, scale=-1.0))
                        g_y.append(ygt)
                        g_av.append(avt)
                        g_s.append(sgt)

                # ---- blend phase (GpSimd + DVE) ------------------------
                wts = {}
                for k in grp:
                    st, yt = sts[k], yts[k]
                    wt = wp.tile([P, F], f16, tag="w")
                    nc.vector.tensor_scalar(wt[:], st[:], CLIP, 1.0,
                                            Alu.add, Alu.min)
                    ct = cp.tile([P, F], f16, tag="c")
                    nc.vector.tensor_scalar(ct[:], st[:], -1.0, 1.0,
                                            Alu.mult, Alu.add)
                    nc.vector.copy_predicated(
                        wt[:], yt[:].bitcast(i32), ct[:])
                    wts[k] = wt
                if gi == 0:
                    for bt in range(NBT):
                        sgt, ygt = g_s[bt], g_y[bt]
                        wgt = gp.tile([P, U_PAD], f32, tag="wg")
                        nc.vector.tensor_scalar(wgt[:], sgt[:], CLIP, 1.0,
                                                Alu.add, Alu.min)
                        cgt = gp.tile([P, U_PAD], f32, tag="cg")
                        nc.vector.tensor_scalar(cgt[:], sgt[:], -1.0, 1.0,
                                                Alu.mult, Alu.add)
                        nc.vector.copy_predicated(
                            wgt[:], ygt[:].bitcast(i32), cgt[:])
                        g_w.append(wgt)

                # ---- Ln phase ------------------------------------------
                for k in grp:
                    wt = wts[k]
                    lt = lp.tile([P, F], f16, tag="lt")
                    act_order(nc.scalar.activation(
                        lt[:], wt[:], Act.Ln, accum_out=accLW[:, k : k + 1]))
                if gi == 0:
                    for bt in range(NBT):
                        lgt = gp.tile([P, U_PAD], f32, tag="lg")
                        act_order(nc.scalar.activation(lgt[:], g_w[bt][:],
                                                       Act.Ln))
                        # accC[:,bt] = sum(-active * loss)
                        ja = gp.tile([P, U_PAD], f32, tag="junk")
                        nc.vector.scalar_tensor_tensor(
                            ja[:], g_av[bt][:], -1.0, lgt[:],
                            Alu.mult, Alu.mult,
                            accum_out=accC[:, bt : bt + 1],
                        )

            # ---- final combine -> out [P, 2] ---------------------------
            lossr = finp.tile([P, 1], f32, tag="lossr")
            nc.vector.tensor_reduce(lossr[:], accLW[:], AX.X, Alu.add)
            corrr = finp.tile([P, 1], f32, tag="corrr")
            nc.vector.tensor_reduce(corrr[:], accC[:], AX.X, Alu.add)

            osb = finp.tile([P, 2], f32, tag="osb")
            nc.vector.tensor_copy(out=osb[:, 0:1], in_=lossr[:])
            nc.vector.tensor_copy(out=osb[:, 1:2], in_=corrr[:])
            nc.sync.dma_start(out[:], osb[:])

    nc.compile()
    return nc


def _get_nc():
    global _NC
    if _NC is None:
        _NC = _build_program()
    return _NC


def _ensure_ntff_hook():
    """Register the axon NTFF profile hook if the image's antenv lacks it."""
    import contextlib
    import ctypes
    import types

    try:
        from antenv.axon_hooks import get_axon_ntff_profile_hook  # noqa: F401
        return
    except ImportError:
        pass

    so_path = "/opt/axon/libaxon_pjrt.so"
    try:
        lib = ctypes.CDLL(so_path)
    except OSError:
        return
    if not hasattr(lib, "axon_start_nrt_profile"):
        return
    lib.axon_start_nrt_profile.argtypes = [
        ctypes.POINTER(ctypes.c_int64),
        ctypes.c_size_t,
    ]
    lib.axon_start_nrt_profile.restype = ctypes.c_int64
    lib.axon_stop_nrt_profile.argtypes = [ctypes.c_char_p]
    lib.axon_stop_nrt_profile.restype = ctypes.c_int64

    @contextlib.contextmanager
    def _hook(output_dir, device_ids):
        import jax

        jax.devices()
        if device_ids:
            ids = (ctypes.c_int64 * len(device_ids))(*device_ids)
            rc = lib.axon_start_nrt_profile(ids, len(device_ids))
        else:
            rc = lib.axon_start_nrt_profile(None, 0)
        if rc != 0:
            raise RuntimeError(f"axon_start_nrt_profile rc={rc}")
        try:
            yield
        finally:
            n = lib.axon_stop_nrt_profile(str(output_dir).encode())
            print(f"ntff profile: {n} file(s) written to {output_dir}",
                  file=sys.stderr)

    mod = types.ModuleType("antenv.axon_hooks")
    mod.get_axon_ntff_profile_hook = lambda: _hook
    mod.set_axon_ntff_profile_hook = lambda h: None
    sys.modules["antenv.axon_hooks"] = mod


def kernel(x, y, recycle_ind, donate_ind, compost_ind):
    global LAST_RESULTS
    import concourse.bass_utils as bass_utils

    # Avoid any network artifact upload in the (optional) trace path.
    bass_utils.upload_artifacts = lambda tmpdir: "local://" + tmpdir
    _ensure_ntff_hook()

    x = np.ascontiguousarray(x, dtype=np.float32)
    y = np.ascontiguousarray(y, dtype=np.float32)
    recycle_ind = np.asarray(recycle_ind).astype(np.int64)
    donate_ind = np.asarray(donate_ind).astype(np.int64)
    compost_ind = np.asarray(compost_ind).astype(np.int64)

    # Union of group columns, padded to the fixed program width. Pad
    # columns carry active=0 so they contribute nothing to the correction.
    cols = np.unique(np.concatenate([recycle_ind, donate_ind, compost_ind]))
    u = len(cols)
    assert u <= U_PAD, (u, U_PAD)
    colsp = np.concatenate([cols, np.zeros(U_PAD - u, dtype=cols.dtype)])

    def mask_v(ind):
        v = np.zeros(U_PAD, np.float32)
        v[:u] = np.isin(cols, ind).astype(np.float32)
        return v

    mrv = mask_v(recycle_ind)
    mdv = mask_v(donate_ind)
    mcv = mask_v(compost_ind)

    xg = np.ascontiguousarray(x[:, colsp])
    yg = np.ascontiguousarray(y[:, colsp])

    # active[b, j] from the group masks and per-row has-group flags
    has_r = (yg * mrv).sum(axis=1) > 0
    has_d = (yg * mdv).sum(axis=1) > 0
    has_c = (yg * mcv).sum(axis=1) > 0
    any_g = has_r | has_d | has_c
    a_r = (any_g & ~has_r).astype(np.float32)
    a_d = (any_g & ~has_d).astype(np.float32)
    a_c = (any_g & ~has_c).astype(np.float32)
    av = np.minimum(
        a_r[:, None] * mrv + a_d[:, None] * mdv + a_c[:, None] * mcv, 1.0
    ).astype(np.float32)

    nc = _get_nc()

    in_maps = []
    for i in range(NCORES):
        rs = slice(i * ROWS, (i + 1) * ROWS)
        in_maps.append({
            "x": x[rs].reshape(P, FREE),
            "y": y[rs].reshape(P, FREE),
            "xg": xg[rs],
            "yg": yg[rs],
            "avg": av[rs],
        })

    res = bass_utils.run_bass_kernel_spmd(
        nc, in_maps, core_ids=list(range(NCORES)), trace=TRACE
    )
    LAST_RESULTS = res

    loss_sum = 0.0
    corr_neg = 0.0
    for r in res.results:
        o = r["out"].astype(np.float64)
        loss_sum += o[:, 0].sum()
        corr_neg += o[:, 1].sum()

    total = -loss_sum - (1.0 - ALPHA) * corr_neg
    return np.asarray(total, dtype=np.float32)


# revision 13
# speedup vs baseline: 1.3636x; 1.0632x over previous
"""Trainium2 Bass kernel for nn_AsymmetricLossCustom (8 NeuronCores).

Math (reference):
    s  = sigmoid(x)
    t  = min(1 - s + 0.05, 1)
    loss = y*ln(max(s,eps)) + (1-y)*ln(max(t,eps))        # [B, C]
    active[b,c] = OR_g ( (any_g[b] & ~has_g[b]) & mask_g[c] )
    out = -(loss * where(active, 0.1, 1.0)).sum()

Device scheme (2 ScalarE + 1 GpSimd + 1 GpSimd + 1 VectorE pass / element):
    sp = sigmoid(-x)                     # ACT (sigmoid table set)
    t  = min(sp + 0.05, 1)               # GpSimd tensor_scalar dual-op
    c  = 1 - sp            ( = s )       # GpSimd tensor_scalar dual-op
    w  = y ? c : t                       # DVE copy_predicated (int32 mask view)
    loss = ln(w)                         # ACT (natural_log set), fused
                                         #   accum_out => per-row sum(loss)

Sigmoid and ln live in different ACT table sets, so chunks are processed in
groups: all sigmoids of a group, then all lns — 2 table loads per group
instead of 2 per chunk.

The `active` down-weighting only touches columns appearing in one of the
three index arrays (<=170 of 9605). The host gathers those columns and
builds the 0/1 `active` matrix (pure index/mask logic); the device computes
the loss values for the gathered columns and accumulates
    corr_neg = sum(active * (-loss))
so that  out = -sum(loss) - 0.9 * corr_neg.

Sharding: pure data parallel over the batch. Each core gets 512 rows,
viewed as [128 partitions, 38420 free] (4 rows per partition, contiguous),
plus gathered columns [512, 176]. Host sums the 8 per-core [128, 2]
partials:  result = -sum(out[:,0]) - 0.9 * sum(out[:,1]).
"""

import sys

import numpy as np

if "/opt/trn_rl_repo" not in sys.path:
    sys.path.insert(0, "/opt/trn_rl_repo")

B, C = 4096, 9605
NCORES = 8
ROWS = B // NCORES          # 512 rows per core
P = 128                     # SBUF partitions
FREE = (ROWS // P) * C      # 38420 f32 per partition
NCHUNK = 17
F = FREE // NCHUNK          # 2260 (even: enables DVE 4x mode on bf16 TS ops)
GROUPS = [range(0, 6), range(6, 12), range(12, 17)]
WBUFS = 6                   # covers one phase group
NBT = ROWS // P             # 4 gathered batch-tiles per core
U_PAD = 176                 # padded union-column count (>= 70+70+30)
CLIP = 0.05
ALPHA = 0.1

TRACE = False               # set True (e.g. from test.py) to capture an NTFF profile
LAST_RESULTS = None         # BassKernelResults of the most recent run

_NC = None


def _build_program():
    import concourse.bacc as bacc
    import concourse.mybir as mybir
    from concourse import tile

    from concourse.tile import add_dep_helper

    f32 = mybir.dt.float32
    f16 = mybir.dt.float16
    u8 = mybir.dt.uint8
    i32 = mybir.dt.int32
    Alu = mybir.AluOpType
    Act = mybir.ActivationFunctionType
    AX = mybir.AxisListType

    # Force the ACT engine to execute activations in emission order —
    # otherwise the Tile scheduler interleaves sigmoid and ln chunks and
    # the compiler inserts an ACT_TABLE_LOAD (~1.3us) before nearly every
    # activation instead of one per phase.
    _prev_act = [None]

    def act_order(bi):
        if _prev_act[0] is not None:
            add_dep_helper(bi.ins, _prev_act[0].ins, sync=True,
                           reason="act table-set phase order")
        _prev_act[0] = bi
        return bi

    nc = bacc.Bacc(
        "TRN2",
        target_bir_lowering=False,
        debug=False,
        enable_asserts=False,
        num_devices=NCORES,
    )

    x = nc.dram_tensor("x", [P, FREE], f32, kind="ExternalInput").ap()
    y = nc.dram_tensor("y", [P, FREE], u8, kind="ExternalInput").ap()
    xg = nc.dram_tensor("xg", [ROWS, U_PAD], f32, kind="ExternalInput").ap()
    yg = nc.dram_tensor("yg", [ROWS, U_PAD], f32, kind="ExternalInput").ap()
    avg = nc.dram_tensor("avg", [ROWS, U_PAD], f32, kind="ExternalInput").ap()
    out = nc.dram_tensor("out", [P, 2], f32, kind="ExternalOutput").ap()

    with tile.TileContext(nc) as tc:
        with (
            tc.tile_pool(name="xp", bufs=6) as xp,
            tc.tile_pool(name="yp", bufs=6) as yp,
            tc.tile_pool(name="sp", bufs=2) as sp,
            tc.tile_pool(name="wp", bufs=WBUFS) as wp,
            tc.tile_pool(name="cp", bufs=2) as cp,
            tc.tile_pool(name="lp", bufs=2) as lp,
            tc.tile_pool(name="accp", bufs=1) as accp,
            tc.tile_pool(name="gp", bufs=3) as gp,
            tc.tile_pool(name="finp", bufs=1) as finp,
        ):
            accLW = accp.tile([P, NCHUNK], f32, tag="accLW")
            accC = accp.tile([P, NBT], f32, tag="accC")

            g_y, g_s, g_w, g_av = [], [], [], []

            for gi, grp in enumerate(GROUPS):
                # ---- DMA + sigmoid phase -------------------------------
                yts, sts = {}, {}
                for k in grp:
                    cs = slice(k * F, (k + 1) * F)
                    xt = xp.tile([P, F], f32, tag="x")
                    nc.sync.dma_start(xt[:], x[:, cs])
                    yt = yp.tile([P, F], u8, tag="y")
                    nc.sync.dma_start(yt[:], y[:, cs])
                    st = sp.tile([P, F], f16, tag="s")
                    act_order(nc.scalar.activation(st[:], xt[:], Act.Sigmoid,
                                                   scale=-1.0))
                    yts[k], sts[k] = yt, st
                if gi == 0:
                    for bt in range(NBT):
                        rs = slice(bt * P, (bt + 1) * P)
                        xgt = gp.tile([P, U_PAD], f32, tag="xg")
                        nc.sync.dma_start(xgt[:], xg[rs, :])
                        ygt = gp.tile([P, U_PAD], f32, tag="yg")
                        nc.sync.dma_start(ygt[:], yg[rs, :])
                        avt = gp.tile([P, U_PAD], f32, tag="av")
                        nc.sync.dma_start(avt[:], avg[rs, :])
                        sgt = gp.tile([P, U_PAD], f32, tag="sg")
                        act_order(nc.scalar.activation(sgt[:], xgt[:],
                                                       Act.Sigmoid, scale=-1.0))
                        g_y.append(ygt)
                        g_av.append(avt)
                        g_s.append(sgt)

                # ---- blend phase (GpSimd + DVE) ------------------------
                wts = {}
                for k in grp:
                    st, yt = sts[k], yts[k]
                    wt = wp.tile([P, F], f16, tag="w")
                    nc.vector.tensor_scalar(wt[:], st[:], CLIP, 1.0,
                                            Alu.add, Alu.min)
                    ct = cp.tile([P, F], f16, tag="c")
                    nc.vector.tensor_scalar(ct[:], st[:], -1.0, 1.0,
                                            Alu.mult, Alu.add)
                    nc.vector.copy_predicated(wt[:], yt[:], ct[:])
                    wts[k] = wt
                if gi == 0:
                    for bt in range(NBT):
                        sgt, ygt = g_s[bt], g_y[bt]
                        wgt = gp.tile([P, U_PAD], f32, tag="wg")
                        nc.vector.tensor_scalar(wgt[:], sgt[:], CLIP, 1.0,
                                                Alu.add, Alu.min)
                        cgt = gp.tile([P, U_PAD], f32, tag="cg")
                        nc.vector.tensor_scalar(cgt[:], sgt[:], -1.0, 1.0,
                                                Alu.mult, Alu.add)
                        nc.vector.copy_predicated(
                            wgt[:], ygt[:].bitcast(i32), cgt[:])
                        g_w.append(wgt)

                # ---- Ln phase ------------------------------------------
                for k in grp:
                    wt = wts[k]
                    lt = lp.tile([P, F], f16, tag="lt")
                    act_order(nc.scalar.activation(
                        lt[:], wt[:], Act.Ln, accum_out=accLW[:, k : k + 1]))
                if gi == 0:
                    for bt in range(NBT):
                        lgt = gp.tile([P, U_PAD], f32, tag="lg")
                        act_order(nc.scalar.activation(lgt[:], g_w[bt][:],
                                                       Act.Ln))
                        # accC[:,bt] = sum(-active * loss)
                        ja = gp.tile([P, U_PAD], f32, tag="junk")
                        nc.vector.scalar_tensor_tensor(
                            ja[:], g_av[bt][:], -1.0, lgt[:],
                            Alu.mult, Alu.mult,
                            accum_out=accC[:, bt : bt + 1],
                        )

            # ---- final combine -> out [P, 2] ---------------------------
            lossr = finp.tile([P, 1], f32, tag="lossr")
            nc.vector.tensor_reduce(lossr[:], accLW[:], AX.X, Alu.add)
            corrr = finp.tile([P, 1], f32, tag="corrr")
            nc.vector.tensor_reduce(corrr[:], accC[:], AX.X, Alu.add)

            osb = finp.tile([P, 2], f32, tag="osb")
            nc.vector.tensor_copy(out=osb[:, 0:1], in_=lossr[:])
            nc.vector.tensor_copy(out=osb[:, 1:2], in_=corrr[:])
            nc.sync.dma_start(out[:], osb[:])

    nc.compile()
    return nc


def _get_nc():
    global _NC
    if _NC is None:
        _NC = _build_program()
    return _NC


def _ensure_ntff_hook():
    """Register the axon NTFF profile hook if the image's antenv lacks it."""
    import contextlib
    import ctypes
    import types

    try:
        from antenv.axon_hooks import get_axon_ntff_profile_hook  # noqa: F401
        return
    except ImportError:
        pass

    so_path = "/opt/axon/libaxon_pjrt.so"
    try:
        lib = ctypes.CDLL(so_path)
    except OSError:
        return
    if not hasattr(lib, "axon_start_nrt_profile"):
        return
    lib.axon_start_nrt_profile.argtypes = [
        ctypes.POINTER(ctypes.c_int64),
        ctypes.c_size_t,
    ]
    lib.axon_start_nrt_profile.restype = ctypes.c_int64
    lib.axon_stop_nrt_profile.argtypes = [ctypes.c_char_p]
    lib.axon_stop_nrt_profile.restype = ctypes.c_int64

    @contextlib.contextmanager
    def _hook(output_dir, device_ids):
        import jax

        jax.devices()
        if device_ids:
            ids = (ctypes.c_int64 * len(device_ids))(*device_ids)
            rc = lib.axon_start_nrt_profile(ids, len(device_ids))
        else:
            rc = lib.axon_start_nrt_profile(None, 0)
        if rc != 0:
            raise RuntimeError(f"axon_start_nrt_profile rc={rc}")
        try:
            yield
        finally:
            n = lib.axon_stop_nrt_profile(str(output_dir).encode())
            print(f"ntff profile: {n} file(s) written to {output_dir}",
                  file=sys.stderr)

    mod = types.ModuleType("antenv.axon_hooks")
    mod.get_axon_ntff_profile_hook = lambda: _hook
    mod.set_axon_ntff_profile_hook = lambda h: None
    sys.modules["antenv.axon_hooks"] = mod


def kernel(x, y, recycle_ind, donate_ind, compost_ind):
    global LAST_RESULTS
    import concourse.bass_utils as bass_utils

    # Avoid any network artifact upload in the (optional) trace path.
    bass_utils.upload_artifacts = lambda tmpdir: "local://" + tmpdir
    _ensure_ntff_hook()

    x = np.ascontiguousarray(x, dtype=np.float32)
    y = np.ascontiguousarray(y, dtype=np.float32)
    yu8 = y.astype(np.uint8)
    recycle_ind = np.asarray(recycle_ind).astype(np.int64)
    donate_ind = np.asarray(donate_ind).astype(np.int64)
    compost_ind = np.asarray(compost_ind).astype(np.int64)

    # Union of group columns, padded to the fixed program width. Pad
    # columns carry active=0 so they contribute nothing to the correction.
    cols = np.unique(np.concatenate([recycle_ind, donate_ind, compost_ind]))
    u = len(cols)
    assert u <= U_PAD, (u, U_PAD)
    colsp = np.concatenate([cols, np.zeros(U_PAD - u, dtype=cols.dtype)])

    def mask_v(ind):
        v = np.zeros(U_PAD, np.float32)
        v[:u] = np.isin(cols, ind).astype(np.float32)
        return v

    mrv = mask_v(recycle_ind)
    mdv = mask_v(donate_ind)
    mcv = mask_v(compost_ind)

    xg = np.ascontiguousarray(x[:, colsp])
    yg = np.ascontiguousarray(y[:, colsp])

    # active[b, j] from the group masks and per-row has-group flags
    has_r = (yg * mrv).sum(axis=1) > 0
    has_d = (yg * mdv).sum(axis=1) > 0
    has_c = (yg * mcv).sum(axis=1) > 0
    any_g = has_r | has_d | has_c
    a_r = (any_g & ~has_r).astype(np.float32)
    a_d = (any_g & ~has_d).astype(np.float32)
    a_c = (any_g & ~has_c).astype(np.float32)
    av = np.minimum(
        a_r[:, None] * mrv + a_d[:, None] * mdv + a_c[:, None] * mcv, 1.0
    ).astype(np.float32)

    nc = _get_nc()

    in_maps = []
    for i in range(NCORES):
        rs = slice(i * ROWS, (i + 1) * ROWS)
        in_maps.append({
            "x": x[rs].reshape(P, FREE),
            "y": yu8[rs].reshape(P, FREE),
            "xg": xg[rs],
            "yg": yg[rs],
            "avg": av[rs],
        })

    res = bass_utils.run_bass_kernel_spmd(
        nc, in_maps, core_ids=list(range(NCORES)), trace=TRACE
    )
    LAST_RESULTS = res

    loss_sum = 0.0
    corr_neg = 0.0
    for r in res.results:
        o = r["out"].astype(np.float64)
        loss_sum += o[:, 0].sum()
        corr_neg += o[:, 1].sum()

    total = -loss_sum - (1.0 - ALPHA) * corr_neg
    return np.asarray(total, dtype=np.float32)
